# revision 44
# baseline (speedup 1.0000x reference)
"""DGCNN (4x DynamicEdgeConv + global head) Trainium2 Bass kernel.

Sharding: data-parallel over the 16 clouds -> 8 cores x 2 clouds.
Everything on-device per core except the tiny BatchNorm head (needs
cross-core batch stats), computed on host from the per-core h = lin1(pooled)
rows (16x1024 total, ~0.1% of the FLOPs).

Per cloud/layer (all feature-major [C, N] so the PE contracts partitions):
  s[i,j]  = x_i . x_j - 0.5|x_j|^2   (fp32 PE matmul, fused bias row)
  topk    = mantissa-packed trick: packed = (s & ~0x3FF) | j, then
            3x max8 + 2x match_replace on the DVE -> top-20 j per row exactly
            (within 2^-13 relative score granularity; validated vs reference)
  gather  = GPSIMD ap_gather of V.T columns (edge order e = p' + 16*(c'*20+k))
  h1      = relu(U.T[:,i] + Vg.T)  (DVE add with step-0 broadcast + ACT relu)
  h2      = Wb.T @ h1 in PSUM (f32r matmuls), max over the 20 slots via a
            strided DVE tensor_reduce straight out of PSUM, + bias.
"""
import numpy as np

import concourse.bacc as bacc
import concourse.mybir as mybir
import concourse.tile as tile
from concourse.bass_utils import run_bass_kernel_spmd

dt = mybir.dt
AF = mybir.ActivationFunctionType
OP = mybir.AluOpType
AX = mybir.AxisListType

B, N, K = 16, 1024, 20
NCORES = 8
CLOUDS_PER_CORE = B // NCORES
LAYERS = [(3, 64), (64, 64), (64, 128), (128, 256)]  # (Cin, Cout)
ECHUNK = 1280            # edges per gather/h1 chunk (= 64 points * 20)
PCHUNK = 640             # edges per h2 psum tile (= 32 points * 20)
import os
F32R_H2 = os.environ.get("KK_F32R", "1") == "1"  # h2 matmuls in float32r
DEBUG_DUMP = os.environ.get("KK_DEBUG", "0") == "1"

_NC_CACHE = {}


def _mm_dt(ap):
    return ap.bitcast(dt.float32r) if F32R_H2 else ap


def _build_nc():
    if "nc" in _NC_CACHE:
        return _NC_CACHE["nc"]
    nc = bacc.Bacc("TRN2", target_bir_lowering=False, debug=False,
                   num_devices=NCORES)

    # ---------------- DRAM I/O ----------------
    posT = nc.dram_tensor("posT", [CLOUDS_PER_CORE, 3, N], dt.float32,
                          kind="ExternalInput").ap()
    wau, wav, wba, wb, wbb = [], [], [], [], []
    for li, (C, Co) in enumerate(LAYERS):
        wau.append(nc.dram_tensor(f"wau{li}", [C, Co], dt.float32, kind="ExternalInput").ap())
        wav.append(nc.dram_tensor(f"wav{li}", [C, Co], dt.float32, kind="ExternalInput").ap())
        wba.append(nc.dram_tensor(f"wba{li}", [Co, 1], dt.float32, kind="ExternalInput").ap())
        wb.append(nc.dram_tensor(f"wb{li}", [Co, Co], dt.float32, kind="ExternalInput").ap())
        wbb.append(nc.dram_tensor(f"wbb{li}", [Co, 1], dt.float32, kind="ExternalInput").ap())
    lin1 = nc.dram_tensor("lin1", [512, 1024], dt.float32, kind="ExternalInput").ap()
    lin1b = nc.dram_tensor("lin1b", [128, 8], dt.float32, kind="ExternalInput").ap()
    h_out = nc.dram_tensor("h_out", [128, 8 * CLOUDS_PER_CORE], dt.float32,
                           kind="ExternalOutput").ap()
    dbg = {}
    if DEBUG_DUMP:
        for li, (C, Co) in enumerate(LAYERS):
            dbg[f"x{li + 1}"] = nc.dram_tensor(f"dbg_x{li + 1}", [Co, N], dt.float32,
                                               kind="ExternalOutput").ap()
        dbg["idx1"] = nc.dram_tensor("dbg_idx1", [128, 8, 20], dt.int16,
                                     kind="ExternalOutput").ap()
        dbg["pooled"] = nc.dram_tensor("dbg_pooled", [128, 8], dt.float32,
                                       kind="ExternalOutput").ap()
        dbg["vg"] = nc.dram_tensor("dbg_vg", [64, ECHUNK], dt.float32,
                                   kind="ExternalOutput").ap()
        dbg["h1"] = nc.dram_tensor("dbg_h1", [64, ECHUNK], dt.float32,
                                   kind="ExternalOutput").ap()
        dbg["h2"] = nc.dram_tensor("dbg_h2", [64, 1024], dt.float32,
                                   kind="ExternalOutput").ap()
        dbg["ut"] = nc.dram_tensor("dbg_ut", [64, N], dt.float32,
                                   kind="ExternalOutput").ap()
        dbg["vt"] = nc.dram_tensor("dbg_vt", [64, N], dt.float32,
                                   kind="ExternalOutput").ap()

    with tile.TileContext(nc) as tc:
        with (
            tc.tile_pool(name="const", bufs=1) as cpool,
            tc.tile_pool(name="xf", bufs=1) as xpool,          # per-layer features
            tc.tile_pool(name="uv", bufs=2) as uvpool,         # U.T / V.T tiles
            tc.tile_pool(name="pk", bufs=2) as pkpool,         # packed topk tiles
            tc.tile_pool(name="small", bufs=2) as smpool,
            tc.tile_pool(name="idx", bufs=2) as idxpool,
            tc.tile_pool(name="vg", bufs=2) as vgpool,         # gathered V / h1 chunks
            tc.tile_pool(name="ps_s", bufs=2, space="PSUM") as ps_s,
            tc.tile_pool(name="ps_uv", bufs=1, space="PSUM") as ps_uv,
            tc.tile_pool(name="ps_h2", bufs=2, space="PSUM") as ps_h2,
        ):
            # ------------- constants -------------
            iota = cpool.tile([128, N], dt.int32, tag="iota")
            nc.gpsimd.iota(iota[:], pattern=[[1, N]], base=0, channel_multiplier=0)
            maskhi = cpool.tile([128, 1], dt.int32, tag="maskhi")
            nc.vector.memset(maskhi[:], -1024)          # 0xFFFFFC00
            masklo = cpool.tile([128, 1], dt.int32, tag="masklo")
            nc.vector.memset(masklo[:], 1023)           # 0x000003FF
            ones1 = cpool.tile([1, 128], dt.float32, tag="ones1")
            nc.vector.memset(ones1[:], 1.0)

            # weights to SBUF
            wau_sb, wav_sb, wba_sb, wb_sb, wbb_sb = [], [], [], [], []
            for li, (C, Co) in enumerate(LAYERS):
                t = cpool.tile([C, Co], dt.float32, tag=f"wau{li}")
                nc.sync.dma_start(t[:], wau[li][:]); wau_sb.append(t)
                t = cpool.tile([C, Co], dt.float32, tag=f"wav{li}")
                nc.sync.dma_start(t[:], wav[li][:]); wav_sb.append(t)
                nkb = (Co + 127) // 128
                bblks = []
                for ob in range(nkb):
                    kk = min(128, Co - ob * 128)
                    t = cpool.tile([kk, 1], dt.float32, tag=f"wba{li}_{ob}")
                    nc.sync.dma_start(t[:], wba[li][ob * 128: ob * 128 + kk, :])
                    bblks.append(t)
                wba_sb.append(bblks)
                # Wb as [nkb, 128, Co] row-blocks (lhsT K-blocks), f32r for fast PE
                wdt = dt.float32r if F32R_H2 else dt.float32
                blks = []
                for kb in range(nkb):
                    kk = min(128, Co - kb * 128)
                    t = cpool.tile([kk, Co], wdt, tag=f"wb{li}_{kb}")
                    nc.sync.dma_start(t[:], wb[li][kb * 128: kb * 128 + kk, :].bitcast(wdt))
                    blks.append(t)
                wb_sb.append(blks)
                bblks = []
                for ob in range(nkb):
                    kk = min(128, Co - ob * 128)
                    t = cpool.tile([kk, 1], dt.float32, tag=f"wbb{li}_{ob}")
                    nc.sync.dma_start(t[:], wbb[li][ob * 128: ob * 128 + kk, :])
                    bblks.append(t)
                wbb_sb.append(bblks)
            lin1_sb = []
            for kb in range(4):
                t = cpool.tile([128, 1024], dt.float32, tag=f"lin1_{kb}")
                nc.sync.dma_start(t[:], lin1[kb * 128:(kb + 1) * 128, :])
                lin1_sb.append(t)
            lin1b_sb = cpool.tile([128, 8], dt.float32, tag="lin1b")
            nc.sync.dma_start(lin1b_sb[:], lin1b[:])

            pooled = cpool.tile([128, 4 * CLOUDS_PER_CORE], dt.float32, tag="pooled")
            # pooled[:, 4*cl + kb] = kstep kb of cloud cl's 512-dim pooled vec
            # layout: kb0 = [x1(64); x2(64)], kb1 = x3, kb2 = x4[:128], kb3 = x4[128:]

            for cl in range(CLOUDS_PER_CORE):
                # features X.T per layer; layer 0 input from DRAM (pre-transposed)
                # layers with C < 128 carry sqh as an extra partition row
                # (tile [C+1, N], row C = -0.5|x_j|^2) so the gram matmul can
                # take rhs=[X; sqh] in one K=C+1 call.
                xts = []   # list of lists (partition blocks of <=128)
                x0 = xpool.tile([3, N], dt.float32, tag="xA_0")
                nc.sync.dma_start(x0[:], posT[cl])
                xts.append([x0])

                for li, (C, Co) in enumerate(LAYERS):
                    xt_blocks = xts[li]      # current input blocks
                    nkb_in = (C + 127) // 128
                    fused_sq = (C == 64)     # input tiles carry the sqh row (32-aligned)
                    cbs = [min(128, C - kb * 128) for kb in range(nkb_in)]

                    # ---- sqh = -0.5 * sum_f x^2 ----
                    if fused_sq:
                        sqh = xt_blocks[0][C:C + 1, :]
                    else:
                        sqh_t = smpool.tile([1, N], dt.float32, tag="sqh")
                        sqh = sqh_t[:]
                    xsqs = []
                    for kb, xb in enumerate(xt_blocks):
                        xsq = smpool.tile([cbs[kb], N], dt.float32, tag=f"xsq{kb}",
                                          name=f"xsq{kb}")
                        nc.scalar.activation(xsq[:], xb[0:cbs[kb], :], AF.Square)
                        xsqs.append(xsq)
                    nh = smpool.tile([128, 1], dt.float32, tag="neghalf")
                    nc.vector.memset(nh[:], -0.5)
                    for j in range(2):
                        sl = slice(j * 512, (j + 1) * 512)
                        sq_ps = ps_uv.tile([1, 512], dt.float32, tag="sqps")
                        for kb, xsq in enumerate(xsqs):
                            nc.tensor.matmul(sq_ps[:], nh[0:xsq.shape[0], :], xsq[:, sl],
                                             start=(kb == 0), stop=(kb == nkb_in - 1))
                        nc.scalar.activation(sqh[:, sl], sq_ps[:], AF.Identity)

                    if fused_sq:
                        # staged lhsT [X; ones] for the single-call gram
                        xg = smpool.tile([C + 1, N], dt.float32, tag="xg")
                        nc.vector.memset(xg[:], 1.0)
                        nc.scalar.activation(xg[0:C, :], xt_blocks[0][0:C, :], AF.Copy)

                    # ---- V.T / U.T ----
                    nkb_out = (Co + 127) // 128
                    vts, uts = [], []
                    for ob in range(nkb_out):
                        Cob = min(128, Co - ob * 128)
                        vt = uvpool.tile([Cob, N], dt.float32, tag=f"vt{ob}")
                        ut = uvpool.tile([Cob, N], dt.float32, tag=f"ut{ob}")
                        for j in range(2):
                            sl = slice(j * 512, (j + 1) * 512)
                            vp = ps_uv.tile([Cob, 512], dt.float32, tag="uvps")
                            for kb, xb in enumerate(xt_blocks):
                                nc.tensor.matmul(
                                    vp[:], wav_sb[li][kb * 128: kb * 128 + cbs[kb],
                                                      ob * 128: ob * 128 + Cob],
                                    xb[0:cbs[kb], sl], start=(kb == 0), stop=(kb == nkb_in - 1))
                            nc.scalar.activation(vt[:, sl], vp[:], AF.Identity)
                            up = ps_uv.tile([Cob, 512], dt.float32, tag="uvps")
                            for kb, xb in enumerate(xt_blocks):
                                nc.tensor.matmul(
                                    up[:], wau_sb[li][kb * 128: kb * 128 + cbs[kb],
                                                      ob * 128: ob * 128 + Cob],
                                    xb[0:cbs[kb], sl], start=(kb == 0), stop=(kb == nkb_in - 1))
                            nc.scalar.activation(ut[:, sl], up[:], AF.Identity,
                                                 bias=wba_sb[li][ob][0:Cob, :])
                        vts.append(vt)
                        uts.append(ut)

                    # ---- gram + packed topk per 128-row block ----
                    idx16 = idxpool.tile([128, 8, 20], dt.int16, tag="idx16")
                    for t in range(8):
                        packed = pkpool.tile([128, N], dt.int32, tag="pka")
                        for j in range(2):
                            sl = slice(j * 512, (j + 1) * 512)
                            sp = ps_s.tile([128, 512], dt.float32, tag="sps")
                            if fused_sq:
                                nc.tensor.matmul(sp[:], xg[:, t * 128:(t + 1) * 128],
                                                 xt_blocks[0][:, sl],
                                                 start=True, stop=True)
                            else:
                                for kb, xb in enumerate(xt_blocks):
                                    nc.tensor.matmul(sp[:], xb[0:cbs[kb], t * 128:(t + 1) * 128],
                                                     xb[0:cbs[kb], sl], start=(kb == 0), stop=False)
                                nc.tensor.matmul(sp[:], ones1[:], sqh[:, sl],
                                                 start=False, stop=True)
                            nc.vector.tensor_scalar(packed[:, sl], sp[:].bitcast(dt.int32),
                                                    maskhi[:], None, op0=OP.bitwise_and)
                        nc.vector.tensor_tensor(packed[:], packed[:], iota[:],
                                                op=OP.bitwise_or)
                        pa = packed[:].bitcast(dt.float32)
                        vals = smpool.tile([128, 24], dt.float32, tag="vals")
                        pb = pkpool.tile([128, N], dt.float32, tag="pkb")
                        pc = pkpool.tile([128, N], dt.float32, tag="pka")
                        nc.vector.max(vals[:, 0:8], pa)
                        nc.vector.match_replace(pb[:], vals[:, 0:8], pa, -3.0e38)
                        nc.vector.max(vals[:, 8:16], pb[:])
                        nc.vector.match_replace(pc[:], vals[:, 8:16], pb[:], -3.0e38)
                        nc.vector.max(vals[:, 16:24], pc[:])
                        idx32 = smpool.tile([128, 20], dt.int32, tag="idx32")
                        nc.vector.tensor_scalar(idx32[:], vals[:, 0:20].bitcast(dt.int32),
                                                masklo[:], None, op0=OP.bitwise_and)
                        nc.vector.tensor_copy(idx16[:, t, :], idx32[:])

                    # ---- rewrap indices to ap_gather layout ----
                    # idxw[p', 160t + 20c + k] = idx16[16c+p', t, k]
                    idxw = idxpool.tile([128, 8 * 160], dt.int16, tag="idxw")
                    idxw_v = idxw[0:16, :].rearrange("p (t c k) -> p t c k", t=8, c=8, k=20)
                    for c in range(8):
                        nc.sync.dma_start(idxw_v[:, :, c, :], idx16[16 * c:16 * (c + 1), :, :])
                    for g in range(1, 8):
                        nc.sync.dma_start(idxw[16 * g:16 * (g + 1), :], idxw[0:16, :])

                    # ---- gather + h1 + h2 + maxK over edge chunks ----
                    par = "B" if li % 2 == 0 else "A"
                    xnext = [xpool.tile(
                        [min(128, Co - ob * 128) + (1 if Co == 64 else 0), N],
                        dt.float32, tag=f"x{par}_{ob}",
                        name=f"x{li + 1}_{cl}_{ob}")
                             for ob in range(nkb_out)]
                    npts = ECHUNK // K               # points per chunk
                    ncg = npts // 16                 # 16-point groups per chunk
                    nch = (N * K) // ECHUNK
                    h1dt = dt.float32r if F32R_H2 else dt.float32
                    for ch in range(nch):
                        ssl = slice(ch * (ECHUNK // 16), (ch + 1) * (ECHUNK // 16))
                        h1s = []
                        for ob in range(nkb_out):
                            Cob = vts[ob].shape[0]
                            vg = vgpool.tile([Cob, ECHUNK], dt.float32, tag=f"vg{ob}",
                                             name=f"vg{ob}")
                            nc.gpsimd.ap_gather(vg[:], vts[ob][:], idxw[0:Cob, ssl],
                                                channels=Cob, num_elems=N, d=1,
                                                num_idxs=ECHUNK)
                            if DEBUG_DUMP and cl == 0 and li == 0 and ch == 0:
                                nc.sync.dma_start(dbg["vg"][:], vg[:])
                                nc.sync.dma_start(dbg["ut"][:], uts[0][:])
                                nc.sync.dma_start(dbg["vt"][:], vts[0][:])
                            # h1 = relu(vg + U.T broadcast), rounded for the matmul
                            ub = (uts[ob][:, ch * npts:(ch + 1) * npts]
                                  .rearrange("f (c p) -> f c p", c=ncg)
                                  .unsqueeze(2).to_broadcast([Cob, ncg, 20, 16]))
                            vgv = vg[:].rearrange("f (c k p) -> f c k p", c=ncg, k=20, p=16)
                            nc.vector.tensor_tensor(vgv, vgv, ub, op=OP.add)
                            h1r = vgpool.tile([Cob, ECHUNK], h1dt, tag=f"h1r{ob}",
                                              name=f"h1r{ob}")
                            nc.scalar.activation(h1r[:], vg[:], AF.Relu)
                            if DEBUG_DUMP and cl == 0 and li == 0 and ch == 0:
                                nc.sync.dma_start(dbg["h1"][:], h1r[:].bitcast(dt.float32))
                            h1s.append(h1r)
                        # h2 psum chunks of 640 edges (=32 points) each
                        for sub in range(ECHUNK // PCHUNK):
                            base = sub * PCHUNK
                            for ob2 in range(nkb_out):
                                Cob2 = min(128, Co - ob2 * 128)
                                # two 320-edge matmuls, each in its own PSUM bank
                                hp = ps_h2.tile([Cob2, 1024], dt.float32, tag="h2ps")
                                for bi, pr in enumerate(((0, 320), (320, PCHUNK))):
                                    for kb in range(nkb_out):
                                        nc.tensor.matmul(
                                            hp[:, bi * 512: bi * 512 + 320],
                                            wb_sb[li][kb][:, ob2 * 128: ob2 * 128 + Cob2],
                                            _mm_dt(h1s[kb][:, base + pr[0]: base + pr[1]]),
                                            start=(kb == 0), stop=(kb == nkb_out - 1))
                                if DEBUG_DUMP and cl == 0 and li == 0 and ch == 0 and sub == 0:
                                    h2tmp = smpool.tile([Cob2, 1024], dt.float32, tag="h2tmp")
                                    nc.scalar.activation(h2tmp[:], hp[:], AF.Copy)
                                    nc.sync.dma_start(dbg["h2"][:], h2tmp[:])
                                # maxK: e = p' + 16*(c*20+k) -> reduce k per bank view
                                hv = (hp[:].rearrange("f (b q) -> f b q", b=2)[:, :, 0:320]
                                      .rearrange("f b (k p) -> f b p k", k=20, p=16))
                                pt0 = ch * npts + sub * 32
                                nc.vector.tensor_reduce(
                                    xnext[ob2][0:Cob2, pt0:pt0 + 32]
                                    .rearrange("f (c p) -> f c p", c=2),
                                    hv, axis=AX.X, op=OP.max)

                    # bias + pool
                    for ob in range(nkb_out):
                        Cob = min(128, Co - ob * 128)
                        nc.vector.tensor_scalar(xnext[ob][0:Cob, :], xnext[ob][0:Cob, :],
                                                wbb_sb[li][ob][0:Cob, :],
                                                None, op0=OP.add)
                        # global max pool into pooled layout
                        if li == 0:
                            dst = pooled[0:64, 4 * cl: 4 * cl + 1]
                        elif li == 1:
                            dst = pooled[64:128, 4 * cl: 4 * cl + 1]
                        elif li == 2:
                            dst = pooled[0:128, 4 * cl + 1: 4 * cl + 2]
                        else:
                            dst = pooled[0:128, 4 * cl + 2 + ob: 4 * cl + 3 + ob]
                        nc.vector.tensor_reduce(dst, xnext[ob][0:Cob, :], axis=AX.X, op=OP.max)
                        if DEBUG_DUMP and cl == 0:
                            nc.sync.dma_start(dbg[f"x{li + 1}"][ob * 128: ob * 128 + Cob, :],
                                              xnext[ob][0:Cob, :])
                    if DEBUG_DUMP and cl == 0 and li == 0:
                        nc.sync.dma_start(dbg["idx1"][:], idx16[:])
                    xts.append(xnext)

            # ---------------- head: h = pooled @ lin1 + lin1_b ----------------
            h_sb = cpool.tile([128, 8, CLOUDS_PER_CORE], dt.float32, tag="h_sb")
            for pb_ in range(8):
                hp = ps_s.tile([128, CLOUDS_PER_CORE], dt.float32, tag="sps")
                for kb in range(4):
                    rhs = pooled[:, :].rearrange("f (c k) -> f k c", c=CLOUDS_PER_CORE)[:, kb, :]
                    nc.tensor.matmul(hp[:], lin1_sb[kb][:, pb_ * 128:(pb_ + 1) * 128],
                                     rhs, start=(kb == 0), stop=(kb == 3))
                nc.scalar.activation(h_sb[:, pb_, :], hp[:], AF.Identity,
                                     bias=lin1b_sb[:, pb_:pb_ + 1])
            # DMA out in SBUF layout [p, pb, cl]; host untangles
            nc.sync.dma_start(h_out[:, :], h_sb[:])
            if DEBUG_DUMP:
                nc.sync.dma_start(dbg["pooled"][:], pooled[:])

    nc.compile()
    _NC_CACHE["nc"] = nc
    return nc


def _prep_inputs(pos):
    """host-side weight/pos prep shared across cores"""
    pos = np.ascontiguousarray(pos.reshape(B, N, 3).transpose(0, 2, 1), dtype=np.float32)
    return pos  # [B, 3, N]


def kernel(**inputs):
    pos = np.asarray(inputs["pos"], np.float32)
    posT = _prep_inputs(pos)

    common = {}
    for li in range(4):
        C, Co = LAYERS[li]
        Wa = np.asarray(inputs[f"W{li + 1}a"], np.float32)
        ba = np.asarray(inputs[f"b{li + 1}a"], np.float32)
        Wb_ = np.asarray(inputs[f"W{li + 1}b"], np.float32)
        bb_ = np.asarray(inputs[f"b{li + 1}b"], np.float32)
        common[f"wau{li}"] = np.ascontiguousarray(Wa[:C] - Wa[C:])
        common[f"wav{li}"] = np.ascontiguousarray(Wa[C:])
        common[f"wba{li}"] = np.ascontiguousarray(ba[:, None])
        common[f"wb{li}"] = Wb_
        common[f"wbb{li}"] = np.ascontiguousarray(bb_[:, None])
    common["lin1"] = np.asarray(inputs["lin1_w"], np.float32)
    common["lin1b"] = np.ascontiguousarray(
        np.asarray(inputs["lin1_b"], np.float32).reshape(8, 128).T)

    nc = _build_nc()
    in_maps = []
    for c in range(NCORES):
        m = dict(common)
        m["posT"] = np.ascontiguousarray(posT[c * CLOUDS_PER_CORE:(c + 1) * CLOUDS_PER_CORE])
        in_maps.append(m)
    res = run_bass_kernel_spmd(nc, in_maps, core_ids=list(range(NCORES)))
    global _LAST_RES
    _LAST_RES = res
    h = np.concatenate(
        [r["h_out"].reshape(128, 8, CLOUDS_PER_CORE).transpose(2, 1, 0).reshape(CLOUDS_PER_CORE, 1024)
         for r in res.results], 0)   # [16, 1024]

    # host head: BN (cross-batch) + relu + lin2 + log_softmax (fp32)
    gamma = np.asarray(inputs["gamma"], np.float32)
    beta = np.asarray(inputs["beta"], np.float32)
    lin2_w = np.asarray(inputs["lin2_w"], np.float32)
    lin2_b = np.asarray(inputs["lin2_b"], np.float32)
    mu = h.mean(0)
    var = ((h - mu) ** 2).mean(0)
    hn = (h - mu) / np.sqrt(var + 1e-5) * gamma + beta
    hn = np.maximum(hn, 0)
    logits = hn @ lin2_w + lin2_b
    m = logits.max(1, keepdims=True)
    lse = np.log(np.exp(logits - m).sum(1, keepdims=True)) + m
    return (logits - lse).astype(np.float32)


# revision 53
# speedup vs baseline: 1.2088x; 1.2088x over previous
"""DGCNN (4x DynamicEdgeConv + global head) Trainium2 Bass kernel.

Sharding: data-parallel over the 16 clouds -> 8 cores x 2 clouds.
Everything on-device per core except the tiny BatchNorm head (needs
cross-core batch stats), computed on host from the per-core h = lin1(pooled)
rows (16x1024 total, ~0.1% of the FLOPs).

Per cloud/layer (all feature-major [C, N] so the PE contracts partitions):
  s[i,j]  = x_i . x_j - 0.5|x_j|^2   (fp32 PE matmul, fused bias row)
  topk    = mantissa-packed trick: packed = (s & ~0x3FF) | j, then
            3x max8 + 2x match_replace on the DVE -> top-20 j per row exactly
            (within 2^-13 relative score granularity; validated vs reference)
  gather  = GPSIMD ap_gather of V.T columns (edge order e = p' + 16*(c'*20+k))
  h1      = relu(U.T[:,i] + Vg.T)  (DVE add with step-0 broadcast + ACT relu)
  h2      = Wb.T @ h1 in PSUM (f32r matmuls), max over the 20 slots via a
            strided DVE tensor_reduce straight out of PSUM, + bias.
"""
import numpy as np

import concourse.bacc as bacc
import concourse.mybir as mybir
import concourse.tile as tile
from concourse.bass_utils import run_bass_kernel_spmd

dt = mybir.dt
AF = mybir.ActivationFunctionType
OP = mybir.AluOpType
AX = mybir.AxisListType

B, N, K = 16, 1024, 20
NCORES = 8
CLOUDS_PER_CORE = B // NCORES
LAYERS = [(3, 64), (64, 64), (64, 128), (128, 256)]  # (Cin, Cout)
ECHUNK = 1280            # edges per gather/h1 chunk (= 64 points * 20)
PCHUNK = 640             # edges per h2 psum tile (= 32 points * 20)
import os
F32R_H2 = os.environ.get("KK_F32R", "1") == "1"  # h2 matmuls in float32r
DEBUG_DUMP = os.environ.get("KK_DEBUG", "0") == "1"

_NC_CACHE = {}


def _mm_dt(ap):
    return ap.bitcast(dt.float32r) if F32R_H2 else ap


def _build_nc():
    if "nc" in _NC_CACHE:
        return _NC_CACHE["nc"]
    nc = bacc.Bacc("TRN2", target_bir_lowering=False, debug=False,
                   num_devices=NCORES)

    # ---------------- DRAM I/O ----------------
    posT = nc.dram_tensor("posT", [CLOUDS_PER_CORE, 3, N], dt.float32,
                          kind="ExternalInput").ap()
    wau, wav, wba, wb, wbb = [], [], [], [], []
    for li, (C, Co) in enumerate(LAYERS):
        wau.append(nc.dram_tensor(f"wau{li}", [C, Co], dt.float32, kind="ExternalInput").ap())
        wav.append(nc.dram_tensor(f"wav{li}", [C, Co], dt.float32, kind="ExternalInput").ap())
        wba.append(nc.dram_tensor(f"wba{li}", [Co, 1], dt.float32, kind="ExternalInput").ap())
        wb.append(nc.dram_tensor(f"wb{li}", [Co, Co], dt.float32, kind="ExternalInput").ap())
        wbb.append(nc.dram_tensor(f"wbb{li}", [Co, 1], dt.float32, kind="ExternalInput").ap())
    lin1 = nc.dram_tensor("lin1", [512, 1024], dt.float32, kind="ExternalInput").ap()
    lin1b = nc.dram_tensor("lin1b", [128, 8], dt.float32, kind="ExternalInput").ap()
    h_out = nc.dram_tensor("h_out", [128, 8 * CLOUDS_PER_CORE], dt.float32,
                           kind="ExternalOutput").ap()
    dbg = {}
    if DEBUG_DUMP:
        for li, (C, Co) in enumerate(LAYERS):
            dbg[f"x{li + 1}"] = nc.dram_tensor(f"dbg_x{li + 1}", [Co, N], dt.float32,
                                               kind="ExternalOutput").ap()
        dbg["idx1"] = nc.dram_tensor("dbg_idx1", [128, 8, 20], dt.int16,
                                     kind="ExternalOutput").ap()
        dbg["pooled"] = nc.dram_tensor("dbg_pooled", [128, 8], dt.float32,
                                       kind="ExternalOutput").ap()
        dbg["vg"] = nc.dram_tensor("dbg_vg", [64, ECHUNK], dt.float32,
                                   kind="ExternalOutput").ap()
        dbg["h1"] = nc.dram_tensor("dbg_h1", [64, ECHUNK], dt.float32,
                                   kind="ExternalOutput").ap()
        dbg["h2"] = nc.dram_tensor("dbg_h2", [64, 1024], dt.float32,
                                   kind="ExternalOutput").ap()
        dbg["ut"] = nc.dram_tensor("dbg_ut", [64, N], dt.float32,
                                   kind="ExternalOutput").ap()
        dbg["vt"] = nc.dram_tensor("dbg_vt", [64, N], dt.float32,
                                   kind="ExternalOutput").ap()

    with tile.TileContext(nc) as tc:
        with (
            tc.tile_pool(name="const", bufs=1) as cpool,
            tc.tile_pool(name="xf", bufs=1) as xpool,          # per-layer features
            tc.tile_pool(name="uv", bufs=2) as uvpool,         # U.T / V.T tiles
            tc.tile_pool(name="pk", bufs=3) as pkpool,         # packed topk tiles
            tc.tile_pool(name="small", bufs=2) as smpool,
            tc.tile_pool(name="idx", bufs=2) as idxpool,
            tc.tile_pool(name="vg", bufs=2) as vgpool,         # gathered V / h1 chunks
            tc.tile_pool(name="ps_s", bufs=2, space="PSUM") as ps_s,
            tc.tile_pool(name="ps_uv", bufs=1, space="PSUM") as ps_uv,
            tc.tile_pool(name="ps_h2", bufs=2, space="PSUM") as ps_h2,
        ):
            # ------------- constants -------------
            iota = cpool.tile([128, N], dt.int32, tag="iota")
            nc.gpsimd.iota(iota[:], pattern=[[1, N]], base=0, channel_multiplier=0)
            maskhi = cpool.tile([128, 1], dt.int32, tag="maskhi")
            nc.vector.memset(maskhi[:], -1024)          # 0xFFFFFC00
            masklo = cpool.tile([128, 1], dt.int32, tag="masklo")
            nc.vector.memset(masklo[:], 1023)           # 0x000003FF
            ones1 = cpool.tile([1, 128], dt.float32, tag="ones1")
            nc.vector.memset(ones1[:], 1.0)

            # weights to SBUF
            wau_sb, wav_sb, wba_sb, wb_sb, wbb_sb = [], [], [], [], []
            for li, (C, Co) in enumerate(LAYERS):
                t = cpool.tile([C, Co], dt.float32, tag=f"wau{li}")
                nc.sync.dma_start(t[:], wau[li][:]); wau_sb.append(t)
                t = cpool.tile([C, Co], dt.float32, tag=f"wav{li}")
                nc.sync.dma_start(t[:], wav[li][:]); wav_sb.append(t)
                nkb = (Co + 127) // 128
                bblks = []
                for ob in range(nkb):
                    kk = min(128, Co - ob * 128)
                    t = cpool.tile([kk, 1], dt.float32, tag=f"wba{li}_{ob}")
                    nc.sync.dma_start(t[:], wba[li][ob * 128: ob * 128 + kk, :])
                    bblks.append(t)
                wba_sb.append(bblks)
                # Wb as [nkb, 128, Co] row-blocks (lhsT K-blocks), f32r for fast PE
                wdt = dt.float32r if F32R_H2 else dt.float32
                blks = []
                for kb in range(nkb):
                    kk = min(128, Co - kb * 128)
                    t = cpool.tile([kk, Co], wdt, tag=f"wb{li}_{kb}")
                    nc.sync.dma_start(t[:], wb[li][kb * 128: kb * 128 + kk, :].bitcast(wdt))
                    blks.append(t)
                wb_sb.append(blks)
                bblks = []
                for ob in range(nkb):
                    kk = min(128, Co - ob * 128)
                    t = cpool.tile([kk, 1], dt.float32, tag=f"wbb{li}_{ob}")
                    nc.sync.dma_start(t[:], wbb[li][ob * 128: ob * 128 + kk, :])
                    bblks.append(t)
                wbb_sb.append(bblks)
            lin1_sb = []
            for kb in range(4):
                t = cpool.tile([128, 1024], dt.float32, tag=f"lin1_{kb}")
                nc.sync.dma_start(t[:], lin1[kb * 128:(kb + 1) * 128, :])
                lin1_sb.append(t)
            lin1b_sb = cpool.tile([128, 8], dt.float32, tag="lin1b")
            nc.sync.dma_start(lin1b_sb[:], lin1b[:])

            pooled = cpool.tile([128, 4 * CLOUDS_PER_CORE], dt.float32, tag="pooled")
            # pooled[:, 4*cl + kb] = kstep kb of cloud cl's 512-dim pooled vec
            # layout: kb0 = [x1(64); x2(64)], kb1 = x3, kb2 = x4[:128], kb3 = x4[128:]

            # layer-major over both clouds so independent per-cloud work can
            # overlap across engines; layers with C == 64 carry sqh as an
            # extra partition row (tile [C+1, N], row C = -0.5|x_j|^2) so the
            # gram matmul can take rhs=[X; sqh] in one K=C+1 call.
            xts_c = {}
            for cl in range(CLOUDS_PER_CORE):
                x0 = xpool.tile([3, N], dt.float32, tag=f"xA0_c{cl}", name=f"x0c{cl}")
                nc.sync.dma_start(x0.copy()[:], posT[cl])
                xts_c[cl] = [[x0]]

            for li, (C, Co) in enumerate(LAYERS):
                for cl in range(CLOUDS_PER_CORE):
                    xts = xts_c[cl]
                    xt_blocks = xts[li]      # current input blocks
                    nkb_in = (C + 127) // 128
                    fused_sq = (C == 64)     # input tiles carry the sqh row (32-aligned)
                    cbs = [min(128, C - kb * 128) for kb in range(nkb_in)]

                    # ---- sqh = -0.5 * sum_f x^2 ----
                    if fused_sq:
                        sqh = xt_blocks[0][C:C + 1, :]
                    else:
                        sqh_t = smpool.tile([1, N], dt.float32, tag="sqh")
                        sqh = sqh_t[:]
                    xsqs = []
                    for kb, xb in enumerate(xt_blocks):
                        xsq = smpool.tile([cbs[kb], N], dt.float32, tag=f"xsq{kb}",
                                          name=f"xsq{kb}")
                        nc.scalar.activation(xsq[:], xb[0:cbs[kb], :], AF.Square)
                        xsqs.append(xsq)
                    nh = smpool.tile([128, 1], dt.float32, tag="neghalf")
                    nc.vector.memset(nh[:], -0.5)
                    for j in range(2):
                        sl = slice(j * 512, (j + 1) * 512)
                        sq_ps = ps_uv.tile([1, 512], dt.float32, tag="sqps")
                        for kb, xsq in enumerate(xsqs):
                            nc.tensor.matmul(sq_ps[:], nh[0:xsq.shape[0], :], xsq[:, sl],
                                             start=(kb == 0), stop=(kb == nkb_in - 1))
                        nc.scalar.activation(sqh[:, sl], sq_ps[:], AF.Identity)

                    if fused_sq:
                        # staged lhsT [X; ones] for the single-call gram
                        xg = smpool.tile([C + 1, N], dt.float32, tag=f"xg_c{cl}",
                                         name=f"xg_c{cl}")
                        nc.vector.memset(xg[:], 1.0)
                        nc.scalar.activation(xg[0:C, :], xt_blocks[0][0:C, :], AF.Copy)

                    # ---- V.T / U.T ----
                    nkb_out = (Co + 127) // 128
                    vts, uts = [], []
                    for ob in range(nkb_out):
                        Cob = min(128, Co - ob * 128)
                        vt = uvpool.tile([Cob, N], dt.float32, tag=f"vt{ob}")
                        ut = uvpool.tile([Cob, N], dt.float32, tag=f"ut{ob}")
                        for j in range(2):
                            sl = slice(j * 512, (j + 1) * 512)
                            vp = ps_uv.tile([Cob, 512], dt.float32, tag="uvps")
                            for kb, xb in enumerate(xt_blocks):
                                nc.tensor.matmul(
                                    vp[:], wav_sb[li][kb * 128: kb * 128 + cbs[kb],
                                                      ob * 128: ob * 128 + Cob],
                                    xb[0:cbs[kb], sl], start=(kb == 0), stop=(kb == nkb_in - 1))
                            nc.scalar.activation(vt[:, sl], vp[:], AF.Identity)
                            up = ps_uv.tile([Cob, 512], dt.float32, tag="uvps")
                            for kb, xb in enumerate(xt_blocks):
                                nc.tensor.matmul(
                                    up[:], wau_sb[li][kb * 128: kb * 128 + cbs[kb],
                                                      ob * 128: ob * 128 + Cob],
                                    xb[0:cbs[kb], sl], start=(kb == 0), stop=(kb == nkb_in - 1))
                            nc.scalar.activation(ut[:, sl], up[:], AF.Identity,
                                                 bias=wba_sb[li][ob][0:Cob, :])
                        vts.append(vt)
                        uts.append(ut)

                    # ---- gram + packed topk per 128-row block ----
                    idx16 = idxpool.tile([128, 8, 20], dt.int16, tag=f"idx16_c{cl}",
                                         name=f"idx16_c{cl}")
                    for t in range(8):
                        packed = pkpool.tile([128, N], dt.int32, tag="pka")
                        for j in range(2):
                            sl = slice(j * 512, (j + 1) * 512)
                            sp = ps_s.tile([128, 512], dt.float32, tag="sps")
                            if fused_sq:
                                nc.tensor.matmul(sp[:], xg[:, t * 128:(t + 1) * 128],
                                                 xt_blocks[0][:, sl],
                                                 start=True, stop=True)
                            else:
                                for kb, xb in enumerate(xt_blocks):
                                    nc.tensor.matmul(sp[:], xb[0:cbs[kb], t * 128:(t + 1) * 128],
                                                     xb[0:cbs[kb], sl], start=(kb == 0), stop=False)
                                nc.tensor.matmul(sp[:], ones1[:], sqh[:, sl],
                                                 start=False, stop=True)
                            nc.vector.tensor_scalar(packed[:, sl], sp[:].bitcast(dt.int32),
                                                    maskhi[:], None, op0=OP.bitwise_and)
                        nc.vector.tensor_tensor(packed[:], packed[:], iota[:],
                                                op=OP.bitwise_or)
                        pa = packed[:].bitcast(dt.float32)
                        vals = smpool.tile([128, 24], dt.float32, tag="vals")
                        pb = pkpool.tile([128, N], dt.float32, tag="pkb")
                        pc = pkpool.tile([128, N], dt.float32, tag="pka")
                        nc.vector.max(vals[:, 0:8], pa)
                        nc.vector.match_replace(pb[:], vals[:, 0:8], pa, -3.0e38)
                        nc.vector.max(vals[:, 8:16], pb[:])
                        nc.vector.match_replace(pc[:], vals[:, 8:16], pb[:], -3.0e38)
                        nc.vector.max(vals[:, 16:24], pc[:])
                        idx32 = smpool.tile([128, 20], dt.int32, tag="idx32")
                        nc.vector.tensor_scalar(idx32[:], vals[:, 0:20].bitcast(dt.int32),
                                                masklo[:], None, op0=OP.bitwise_and)
                        nc.vector.tensor_copy(idx16[:, t, :], idx32[:])

                    # ---- rewrap indices to ap_gather layout ----
                    # idxw[p', 160t + 20c + k] = idx16[16c+p', t, k]
                    idxw = idxpool.tile([128, 8 * 160], dt.int16, tag=f"idxw_c{cl}",
                                        name=f"idxw_c{cl}")
                    idxw_v = idxw[0:16, :].rearrange("p (t c k) -> p t c k", t=8, c=8, k=20)
                    for c in range(8):
                        nc.sync.dma_start(idxw_v[:, :, c, :], idx16[16 * c:16 * (c + 1), :, :])
                    for g in range(1, 8):
                        nc.sync.dma_start(idxw[16 * g:16 * (g + 1), :], idxw[0:16, :])

                    # ---- gather + h1 + h2 + maxK over edge chunks ----
                    par = "B" if li % 2 == 0 else "A"
                    xnext = [xpool.tile(
                        [min(128, Co - ob * 128) + (1 if Co == 64 else 0), N],
                        dt.float32, tag=f"x{par}_{ob}_c{cl}",
                        name=f"x{li + 1}_{cl}_{ob}")
                             for ob in range(nkb_out)]
                    npts = ECHUNK // K               # points per chunk
                    ncg = npts // 16                 # 16-point groups per chunk
                    nch = (N * K) // ECHUNK
                    h1dt = dt.float32r if F32R_H2 else dt.float32
                    for ch in range(nch):
                        ssl = slice(ch * (ECHUNK // 16), (ch + 1) * (ECHUNK // 16))
                        h1s = []
                        for ob in range(nkb_out):
                            Cob = vts[ob].shape[0]
                            vg = vgpool.tile([Cob, ECHUNK], dt.float32, tag=f"vg{ob}",
                                             name=f"vg{ob}")
                            nc.gpsimd.ap_gather(vg[:], vts[ob][:], idxw[0:Cob, ssl],
                                                channels=Cob, num_elems=N, d=1,
                                                num_idxs=ECHUNK)
                            if DEBUG_DUMP and cl == 0 and li == 0 and ch == 0:
                                nc.sync.dma_start(dbg["vg"][:], vg[:])
                                nc.sync.dma_start(dbg["ut"][:], uts[0][:])
                                nc.sync.dma_start(dbg["vt"][:], vts[0][:])
                            # h1 = relu(vg + U.T broadcast), rounded for the matmul
                            ub = (uts[ob][:, ch * npts:(ch + 1) * npts]
                                  .rearrange("f (c p) -> f c p", c=ncg)
                                  .unsqueeze(2).to_broadcast([Cob, ncg, 20, 16]))
                            vgv = vg[:].rearrange("f (c k p) -> f c k p", c=ncg, k=20, p=16)
                            # alternate the u+v add between DVE and GPSIMD to
                            # spread load off the bottleneck vector engine
                            addeng = nc.vector if (ch + ob) % 2 == 0 else nc.gpsimd
                            addeng.tensor_tensor(vgv, vgv, ub, op=OP.add)
                            h1r = vgpool.tile([Cob, ECHUNK], h1dt, tag=f"h1r{ob}",
                                              name=f"h1r{ob}")
                            nc.scalar.activation(h1r[:], vg[:], AF.Relu)
                            if DEBUG_DUMP and cl == 0 and li == 0 and ch == 0:
                                nc.sync.dma_start(dbg["h1"][:], h1r[:].bitcast(dt.float32))
                            h1s.append(h1r)
                        # h2 psum chunks of 640 edges (=32 points) each
                        for sub in range(ECHUNK // PCHUNK):
                            base = sub * PCHUNK
                            for ob2 in range(nkb_out):
                                Cob2 = min(128, Co - ob2 * 128)
                                # two 320-edge matmuls, each in its own PSUM bank
                                hp = ps_h2.tile([Cob2, 1024], dt.float32, tag="h2ps")
                                for bi, pr in enumerate(((0, 320), (320, PCHUNK))):
                                    for kb in range(nkb_out):
                                        nc.tensor.matmul(
                                            hp[:, bi * 512: bi * 512 + 320],
                                            wb_sb[li][kb][:, ob2 * 128: ob2 * 128 + Cob2],
                                            _mm_dt(h1s[kb][:, base + pr[0]: base + pr[1]]),
                                            start=(kb == 0), stop=(kb == nkb_out - 1))
                                if DEBUG_DUMP and cl == 0 and li == 0 and ch == 0 and sub == 0:
                                    h2tmp = smpool.tile([Cob2, 1024], dt.float32, tag="h2tmp")
                                    nc.scalar.activation(h2tmp[:], hp[:], AF.Copy)
                                    nc.sync.dma_start(dbg["h2"][:], h2tmp[:])
                                # maxK: e = p' + 16*(c*20+k) -> reduce k per bank view
                                hv = (hp[:].rearrange("f (b q) -> f b q", b=2)[:, :, 0:320]
                                      .rearrange("f b (k p) -> f b p k", k=20, p=16))
                                pt0 = ch * npts + sub * 32
                                nc.vector.tensor_reduce(
                                    xnext[ob2][0:Cob2, pt0:pt0 + 32]
                                    .rearrange("f (c p) -> f c p", c=2),
                                    hv, axis=AX.X, op=OP.max)

                    # bias + pool
                    for ob in range(nkb_out):
                        Cob = min(128, Co - ob * 128)
                        nc.vector.tensor_scalar(xnext[ob][0:Cob, :], xnext[ob][0:Cob, :],
                                                wbb_sb[li][ob][0:Cob, :],
                                                None, op0=OP.add)
                        # global max pool into pooled layout
                        if li == 0:
                            dst = pooled[0:64, 4 * cl: 4 * cl + 1]
                        elif li == 1:
                            dst = pooled[64:128, 4 * cl: 4 * cl + 1]
                        elif li == 2:
                            dst = pooled[0:128, 4 * cl + 1: 4 * cl + 2]
                        else:
                            dst = pooled[0:128, 4 * cl + 2 + ob: 4 * cl + 3 + ob]
                        nc.vector.tensor_reduce(dst, xnext[ob][0:Cob, :], axis=AX.X, op=OP.max)
                        if DEBUG_DUMP and cl == 0:
                            nc.sync.dma_start(dbg[f"x{li + 1}"][ob * 128: ob * 128 + Cob, :],
                                              xnext[ob][0:Cob, :])
                    if DEBUG_DUMP and cl == 0 and li == 0:
                        nc.sync.dma_start(dbg["idx1"][:], idx16[:])
                    xts.append(xnext)

            # ---------------- head: h = pooled @ lin1 + lin1_b ----------------
            h_sb = cpool.tile([128, 8, CLOUDS_PER_CORE], dt.float32, tag="h_sb")
            for pb_ in range(8):
                hp = ps_s.tile([128, CLOUDS_PER_CORE], dt.float32, tag="sps")
                for kb in range(4):
                    rhs = pooled[:, :].rearrange("f (c k) -> f k c", c=CLOUDS_PER_CORE)[:, kb, :]
                    nc.tensor.matmul(hp[:], lin1_sb[kb][:, pb_ * 128:(pb_ + 1) * 128],
                                     rhs, start=(kb == 0), stop=(kb == 3))
                nc.scalar.activation(h_sb[:, pb_, :], hp[:], AF.Identity,
                                     bias=lin1b_sb[:, pb_:pb_ + 1])
            # DMA out in SBUF layout [p, pb, cl]; host untangles
            nc.sync.dma_start(h_out[:, :], h_sb[:])
            if DEBUG_DUMP:
                nc.sync.dma_start(dbg["pooled"][:], pooled[:])

    nc.compile()
    _NC_CACHE["nc"] = nc
    return nc


def _prep_inputs(pos):
    """host-side weight/pos prep shared across cores"""
    pos = np.ascontiguousarray(pos.reshape(B, N, 3).transpose(0, 2, 1), dtype=np.float32)
    return pos  # [B, 3, N]


def kernel(**inputs):
    pos = np.asarray(inputs["pos"], np.float32)
    posT = _prep_inputs(pos)

    common = {}
    for li in range(4):
        C, Co = LAYERS[li]
        Wa = np.asarray(inputs[f"W{li + 1}a"], np.float32)
        ba = np.asarray(inputs[f"b{li + 1}a"], np.float32)
        Wb_ = np.asarray(inputs[f"W{li + 1}b"], np.float32)
        bb_ = np.asarray(inputs[f"b{li + 1}b"], np.float32)
        common[f"wau{li}"] = np.ascontiguousarray(Wa[:C] - Wa[C:])
        common[f"wav{li}"] = np.ascontiguousarray(Wa[C:])
        common[f"wba{li}"] = np.ascontiguousarray(ba[:, None])
        common[f"wb{li}"] = Wb_
        common[f"wbb{li}"] = np.ascontiguousarray(bb_[:, None])
    common["lin1"] = np.asarray(inputs["lin1_w"], np.float32)
    common["lin1b"] = np.ascontiguousarray(
        np.asarray(inputs["lin1_b"], np.float32).reshape(8, 128).T)

    nc = _build_nc()
    in_maps = []
    for c in range(NCORES):
        m = dict(common)
        m["posT"] = np.ascontiguousarray(posT[c * CLOUDS_PER_CORE:(c + 1) * CLOUDS_PER_CORE])
        in_maps.append(m)
    res = run_bass_kernel_spmd(nc, in_maps, core_ids=list(range(NCORES)))
    global _LAST_RES
    _LAST_RES = res
    h = np.concatenate(
        [r["h_out"].reshape(128, 8, CLOUDS_PER_CORE).transpose(2, 1, 0).reshape(CLOUDS_PER_CORE, 1024)
         for r in res.results], 0)   # [16, 1024]

    # host head: BN (cross-batch) + relu + lin2 + log_softmax (fp32)
    gamma = np.asarray(inputs["gamma"], np.float32)
    beta = np.asarray(inputs["beta"], np.float32)
    lin2_w = np.asarray(inputs["lin2_w"], np.float32)
    lin2_b = np.asarray(inputs["lin2_b"], np.float32)
    mu = h.mean(0)
    var = ((h - mu) ** 2).mean(0)
    hn = (h - mu) / np.sqrt(var + 1e-5) * gamma + beta
    hn = np.maximum(hn, 0)
    logits = hn @ lin2_w + lin2_b
    m = logits.max(1, keepdims=True)
    lse = np.log(np.exp(logits - m).sum(1, keepdims=True)) + m
    return (logits - lse).astype(np.float32)
